# revision 14
# baseline (speedup 1.0000x reference)
"""Ensemble MLP surrogate (16 models, 32->64->64->64->8, relu) on 8 TRN2 cores.

Strategy (data-parallel over batch, weights replicated):
  host packs x transposed + 4x row-replicated [128, B/8] per core in fp16;
  feature-on-partition layout with batch streaming as the matmul moving
  operand.  Duo-burst software pipeline with a depth-3 PSUM rotation
  (per-pair [128,2,512] tiles): pairs are processed two at a time so the
  two epilogue engines (ACT/DVE) run concurrently and PE bursts stay
  long.  L1 runs as a 4-way row-packed quad covering both 512-column
  halves; L2/L3 as row-disjoint (0,0)+(64,64) duos; L4 accumulates two
  pairs per 32-column position (odd pair weights in cols 16-31) so ALL
  16 models land in one [128,512] preds tile per half.  Ensemble mean /
  sum-of-squares reduce on the PE via single selector matmuls into a
  merged stats tile (mean rows 0-7, sumsq rows 32-39), interleaved per
  half into the dense pair region; epilogues are greedily balanced
  across Vector and Scalar with measured cost models.
"""

import numpy as np

N_MODELS = 16
IN_DIM = 32
HID = 64
OUT_DIM = 8
BATCH = 131072
N_CORES = 8
B_CORE = BATCH // N_CORES  # 16384
TILE = 512  # matmul moving-operand columns (fp32 PSUM bank limit on out)
DTILE = 2 * TILE  # batch elements per pipeline step ("double tile")
NPAIR = N_MODELS // 2

# wpackr free-dim layout (fp16 matmul operands, 128 partitions)
# L1: 4 row-group replicas (model 2j at rows 0-31/64-95, 2j+1 at 32-63/96-127)
OFF_W1 = 0  # [128, 8, 64]
OFF_W2 = OFF_W1 + NPAIR * 64  # [128, 8, 64] (rows 0-63 model a, 64-127 b)
OFF_W3 = OFF_W2 + NPAIR * 64  # [128, 8, 64] (same canonical layout)
OFF_W4 = OFF_W3 + NPAIR * 64  # [128, 8, 32] pair blockdiag (cols 16-31 zero)
OFF_SELM = OFF_W4 + NPAIR * 32  # [128, 8]  mean selector (1/16)
OFF_SELS = OFF_SELM + 8  # [128, 8]  sumsq selector (1/16; 16/15 in sqrt)
WR = OFF_SELS + 8
# wpackb free-dim layout (fp32 biases)
OFF_B1 = 0  # [128, 8]
OFF_B2 = OFF_B1 + NPAIR  # [128, 8]
OFF_B3 = OFF_B2 + NPAIR  # [128, 8]
OFF_B4 = OFF_B3 + NPAIR  # [128, 1] (single packed preds tile)
WB = OFF_B4 + 1


# measured epilogue op costs (ns) for greedy DVE/ACT load balancing
def _act_psum(fd):
    return (fd + 313) / 1.2


def _act_sbuf(fd):
    return (fd + 350) / 1.2


def _dve_psum(fd):
    return (fd + 190) / 0.96


def _dve_sq16(fd):
    # fp16 SBUF tensor_tensor square, 2x_1P mode
    return (fd / 2 + 120) / 0.96


def pack_inputs(x, W1, b1, W2, b2, W3, b3, W4, b4, b_core=B_CORE, n_cores=N_CORES):
    """Host-side packing. Returns (xt_per_core list, wpackr fp16, wpackb f32)."""
    f32 = np.float32
    x = np.ascontiguousarray(x, dtype=f32)
    wpack = np.zeros((128, WR), f32)
    wpackb = np.zeros((128, WB), f32)

    w1v = wpack[:, OFF_W1 : OFF_W1 + NPAIR * 64].reshape(128, NPAIR, 64)
    w2v = wpack[:, OFF_W2 : OFF_W2 + NPAIR * 64].reshape(128, NPAIR, 64)
    w3v = wpack[:, OFF_W3 : OFF_W3 + NPAIR * 64].reshape(128, NPAIR, 64)
    w4v = wpack[:, OFF_W4 : OFF_W4 + NPAIR * 32].reshape(128, NPAIR, 32)
    for j in range(NPAIR):
        a, b = 2 * j, 2 * j + 1
        for k in range(4):  # L1 row-group k holds model a (k even) / b (k odd)
            w1v[32 * k : 32 * k + 32, j, :] = W1[a if k % 2 == 0 else b]
        w2v[0:HID, j, :] = W2[a]
        w2v[HID:128, j, :] = W2[b]
        w3v[0:HID, j, :] = W3[a]
        w3v[HID:128, j, :] = W3[b]
        soff = 16 * (j % 2)  # odd pair of a position block -> cols 16-31
        w4v[0:HID, j, soff : soff + OUT_DIM] = W4[a]
        w4v[HID:128, j, soff + OUT_DIM : soff + 2 * OUT_DIM] = W4[b]

    selm = wpack[:, OFF_SELM : OFF_SELM + 8]
    sels = wpack[:, OFF_SELS : OFF_SELS + 8]
    b4v = wpackb[:, OFF_B4]
    for j in range(NPAIR):  # preds row = 32*(j//2) + 16*(j%2) + 8*c + o
        for c in range(2):  # model-within-pair
            for o in range(OUT_DIM):
                p = 32 * (j // 2) + 16 * (j % 2) + 8 * c + o
                selm[p, o] = 1.0 / 16.0
                sels[p, o] = 1.0 / 16.0  # exact in fp16; 16/15 applied at sqrt
                b4v[p] = b4[2 * j + c, o]
    for j in range(NPAIR):
        a, b = 2 * j, 2 * j + 1
        wpackb[0:HID, OFF_B1 + j] = b1[a]
        wpackb[HID:128, OFF_B1 + j] = b1[b]
        wpackb[0:HID, OFF_B2 + j] = b2[a]
        wpackb[HID:128, OFF_B2 + j] = b2[b]
        wpackb[0:HID, OFF_B3 + j] = b3[a]
        wpackb[HID:128, OFF_B3 + j] = b3[b]

    wpack16 = wpack.astype(np.float16)
    x16 = x.astype(np.float16)
    xt_per_core = []
    for c in range(n_cores):
        shard = x16[c * b_core : (c + 1) * b_core]  # [b_core, 32]
        xt = np.ascontiguousarray(np.tile(shard.T, (4, 1)))  # [128, b_core]
        xt_per_core.append(xt)
    return xt_per_core, wpack16, wpackb


def _emit(tc, ctx, xt, wr, wb, meant, stdt, b_core):
    import concourse.bass as bass  # noqa: F401
    from concourse import mybir

    nc = tc.nc
    f32 = mybir.dt.float32
    f16 = mybir.dt.float16
    AF = mybir.ActivationFunctionType
    ALU = mybir.AluOpType

    n_dt = b_core // DTILE

    consts = ctx.enter_context(tc.tile_pool(name="consts", bufs=1))
    xp = ctx.enter_context(tc.tile_pool(name="xp", bufs=3))
    hp = [
        ctx.enter_context(tc.tile_pool(name=f"h{i}p", bufs=2)) for i in range(3)
    ]
    prp = ctx.enter_context(tc.tile_pool(name="prp", bufs=4))
    sqp = ctx.enter_context(tc.tile_pool(name="sqp", bufs=4))
    smp = ctx.enter_context(tc.tile_pool(name="smp", bufs=2))  # small stats sbuf
    outp = ctx.enter_context(tc.tile_pool(name="outp", bufs=2))
    # PSUM budget (8 banks): ph tag 3x[128,2,512]=6 banks + p4 tag 2x[128,512]=2
    # (the per-dtile stats tile borrows a "ph" rotation slot)
    ppool = ctx.enter_context(tc.tile_pool(name="ppool", bufs=1, space="PSUM"))

    cw = consts.tile([128, WR], f16)
    nc.sync.dma_start(out=cw, in_=wr)
    cwb = consts.tile([128, WB], f32)
    nc.sync.dma_start(out=cwb, in_=wb)
    w1v = cw[:, OFF_W1 : OFF_W1 + NPAIR * 64].rearrange("p (j f) -> p j f", f=64)
    w2v = cw[:, OFF_W2 : OFF_W2 + NPAIR * 64].rearrange("p (j f) -> p j f", f=64)
    w3v = cw[:, OFF_W3 : OFF_W3 + NPAIR * 64].rearrange("p (j f) -> p j f", f=64)
    w4v = cw[:, OFF_W4 : OFF_W4 + NPAIR * 32].rearrange("p (j f) -> p j f", f=32)
    selm = cw[:, OFF_SELM : OFF_SELM + 8]
    sels = cw[:, OFF_SELS : OFF_SELS + 8]

    # greedy engine balancer (measured-cost) for PSUM->SBUF epilogues
    eng_ns = {"act": 0.0, "dve": 0.0}

    def epilogue(out, in_, bias, relu, eng=None):
        fd = out.free_size()
        if eng is None:
            eng = (
                "act"
                if eng_ns["act"] + _act_psum(fd)
                <= eng_ns["dve"] + _dve_psum(fd)
                else "dve"
            )
        if eng == "act":
            eng_ns["act"] += _act_psum(fd)
            nc.scalar.activation(
                out, in_, AF.Relu if relu else AF.Identity, bias=bias, scale=1.0
            )
        else:
            eng_ns["dve"] += _dve_psum(fd)
            if relu:
                nc.vector.tensor_scalar(
                    out, in_, bias, 0.0, op0=ALU.add, op1=ALU.max
                )
            else:
                nc.vector.tensor_scalar(out, in_, bias, None, op0=ALU.add)

    def psum_square(out, in_):
        # out(SBUF) = in_(PSUM)^2 — ACT only (DVE cannot read PSUM twice)
        fd = out.free_size()
        eng_ns["act"] += _act_psum(fd)
        nc.scalar.activation(out, in_, AF.Square)

    def psum_copy(out, in_):
        fd = out.free_size()
        if eng_ns["act"] + _act_psum(fd) <= eng_ns["dve"] + _dve_psum(fd):
            eng_ns["act"] += _act_psum(fd)
            nc.scalar.copy(out=out, in_=in_)
        else:
            eng_ns["dve"] += _dve_psum(fd)
            nc.vector.tensor_copy(out, in_)

    pend_preds = []
    pend_stats = []

    def emit_preds(x0, p4):
        pr = {}
        for h in range(2):
            prt = prp.tile([128, TILE], f16, tag="pr", name="prt")
            epilogue(prt, p4[h], cwb[:, OFF_B4 : OFF_B4 + 1], relu=False)
            sqt = sqp.tile([128, TILE], f16, tag="sq", name="sqt")
            nc.gpsimd.tensor_mul(sqt, prt, prt)
            pr[h] = (prt, sqt)
        pend_stats.append((x0, pr))

    def emit_stats_half(x0, pr, h):
        # merged stats tile: mean at rows 0-7, sumsq at rows 32-39 (col pos 32)
        prt, sqt = pr[h]
        st = ppool.tile([40, TILE], f32, tag="ph", bufs=3, name="st")
        nc.tensor.matmul(
            out=st[0:8, :], lhsT=selm, rhs=prt, start=True, stop=True,
            tile_position=(0, 0),
        )
        nc.tensor.matmul(
            out=st[32:40, :], lhsT=sels, rhs=sqt, start=True, stop=True,
            tile_position=(0, 32),
        )
        mean_sb = outp.tile([8, TILE], f32, tag="mean")
        psum_copy(mean_sb, st[0:8, :])
        nc.sync.dma_start(
            out=meant[:, x0 + h * TILE : x0 + (h + 1) * TILE], in_=mean_sb
        )
        m2 = smp.tile([8, TILE], f32, tag="m2")
        psum_square(m2, st[0:8, :])
        nvar = smp.tile([8, TILE], f32, tag="nvar")
        # nvar = mean^2 - E[p^2] == -var * 15/16
        nc.vector.tensor_sub(nvar, m2, st[32:40, :])
        eng_ns["dve"] += _dve_psum(TILE)
        std_sb = outp.tile([8, TILE], f32, tag="std")
        nc.scalar.activation(out=std_sb, in_=nvar, func=AF.Sqrt, scale=-16.0 / 15.0)
        eng_ns["act"] += _act_sbuf(TILE)
        nc.sync.dma_start(
            out=stdt[:, x0 + h * TILE : x0 + (h + 1) * TILE], in_=std_sb
        )

    xt_tiles = {}
    p4_by_t = {}

    def stage_l1(t, p):
        xt_t = xt_tiles[t]
        ph1 = ppool.tile([128, 2, TILE], f32, tag="ph", bufs=3, name=f"ph1_{p}")
        for k in range(4):  # row group k: model k%2, half k//2
            nc.tensor.matmul(
                out=ph1[64 * (k % 2) : 64 * (k % 2) + 64, k // 2, :],
                lhsT=w1v[32 * k : 32 * k + 32, p, :],
                rhs=xt_t[32 * k : 32 * k + 32, k // 2, :],
                start=True,
                stop=True,
                tile_position=(32 * k, 64 * (k % 2)),
            )
        h1 = hp[0].tile([128, 2, TILE], f16, tag="h1", name=f"h1_{p}")
        epilogue(
            h1.rearrange("p h n -> p (h n)"),
            ph1.rearrange("p h n -> p (h n)"),
            cwb[:, OFF_B1 + p : OFF_B1 + p + 1],
            relu=True,
        )
        return h1

    def stage_mid(p, hin, wv, off_b, lay):
        ph = ppool.tile([128, 2, TILE], f32, tag="ph", bufs=3, name=f"ph{lay}_{p}")
        for h in range(2):
            nc.tensor.matmul(
                out=ph[0:64, h, :], lhsT=wv[0:64, p, :],
                rhs=hin[0:64, h, :], start=True, stop=True,
                tile_position=(0, 0),
            )
            nc.tensor.matmul(
                out=ph[64:128, h, :], lhsT=wv[64:128, p, :],
                rhs=hin[64:128, h, :], start=True, stop=True,
                tile_position=(64, 64),
            )
        hout = hp[lay - 1].tile(
            [128, 2, TILE], f16, tag=f"h{lay}", name=f"h{lay}_{p}"
        )
        epilogue(
            hout.rearrange("p h n -> p (h n)"),
            ph.rearrange("p h n -> p (h n)"),
            cwb[:, off_b + p : off_b + p + 1],
            relu=True,
        )
        return hout

    def stage_l4(t, p, h3t):
        dd = p // 2
        for h in range(2):
            nc.tensor.matmul(
                out=p4_by_t[t][h][32 * dd : 32 * dd + 32, :],
                lhsT=w4v[:, p, :],
                rhs=h3t[:, h, :],
                start=(p % 2 == 0),
                stop=(p % 2 == 1),
                tile_position=(0, 32 * dd),
            )

    # one continuous skewed pipeline over all n_dt*NPAIR pairs: pair i+1
    # trails pair i by one layer stage, ACROSS double-tile boundaries, so
    # the engines and the PE never drain at a tile seam
    n_pairs = n_dt * NPAIR
    h1d, h2d, h3d = {}, {}, {}
    for i in range(n_pairs + 3):
        t, p = divmod(i, NPAIR)
        if i < n_pairs:
            if p == 0:
                xt_t = xp.tile([128, 2, TILE], f16, tag="xt", name="xt_t")
                nc.gpsimd.dma_start(
                    out=xt_t,
                    in_=xt[:, t * DTILE : (t + 1) * DTILE].rearrange(
                        "p (h n) -> p h n", n=TILE
                    ),
                )
                xt_tiles[t] = xt_t
                p4_by_t[t] = {
                    h: ppool.tile(
                        [128, TILE], f32, tag="p4", bufs=2, name=f"p4_{h}"
                    )
                    for h in range(2)
                }
                if t >= 3:
                    del xt_tiles[t - 3]
            h1d[i] = stage_l1(t, p)
        if 1 <= i and i - 1 < n_pairs:
            j = i - 1
            h2d[j] = stage_mid(j % NPAIR, h1d.pop(j), w2v, OFF_B2, 2)
        if 2 <= i and i - 2 < n_pairs:
            j = i - 2
            h3d[j] = stage_mid(j % NPAIR, h2d.pop(j), w3v, OFF_B3, 3)
        if 3 <= i and i - 3 < n_pairs:
            j = i - 3
            stage_l4(j // NPAIR, j % NPAIR, h3d.pop(j))
            if j % NPAIR == NPAIR - 1:  # tile's L4 fully emitted
                tl = j // NPAIR
                pend_preds.append((tl * DTILE, p4_by_t.pop(tl)))
        # deferred epilogues of the previous tile, tucked into the dense
        # region after this iteration's matmul stages
        if p == 3 and pend_preds:
            x0p, p4p_ = pend_preds.pop(0)
            emit_preds(x0p, p4p_)
        if p == 4 and pend_stats:
            emit_stats_half(pend_stats[0][0], pend_stats[0][1], 0)
        if p == 6 and pend_stats:
            x0p, prs = pend_stats.pop(0)
            emit_stats_half(x0p, prs, 1)

    for x0p, p4p_ in pend_preds:  # drain remaining deferred preds
        emit_preds(x0p, p4p_)
    for x0p, prs in pend_stats:  # drain remaining deferred stats
        emit_stats_half(x0p, prs, 0)
        emit_stats_half(x0p, prs, 1)


def build(b_core=B_CORE, num_devices=N_CORES):
    from contextlib import ExitStack

    import concourse.bacc as bacc
    import concourse.tile as tile
    from concourse import mybir

    f32 = mybir.dt.float32
    f16 = mybir.dt.float16
    nc = bacc.Bacc(
        "TRN2", target_bir_lowering=False, debug=False, num_devices=num_devices
    )
    xt = nc.dram_tensor("xt", [128, b_core], f16, kind="ExternalInput").ap()
    wr = nc.dram_tensor("wpackr", [128, WR], f16, kind="ExternalInput").ap()
    wb = nc.dram_tensor("wpackb", [128, WB], f32, kind="ExternalInput").ap()
    meant = nc.dram_tensor("meant", [8, b_core], f32, kind="ExternalOutput").ap()
    stdt = nc.dram_tensor("stdt", [8, b_core], f32, kind="ExternalOutput").ap()
    with tile.TileContext(nc) as tc:
        with ExitStack() as ctx:
            _emit(tc, ctx, xt, wr, wb, meant, stdt, b_core)
    nc.compile()
    return nc


_NC_CACHE = {}


def kernel(x, W1, b1, W2, b2, W3, b3, W4, b4):
    from concourse.bass_utils import run_bass_kernel_spmd

    key = ("full", B_CORE)
    if key not in _NC_CACHE:
        _NC_CACHE[key] = build(B_CORE, N_CORES)
    nc = _NC_CACHE[key]

    xt_per_core, wpackr, wpackb = pack_inputs(
        np.asarray(x), np.asarray(W1), np.asarray(b1), np.asarray(W2),
        np.asarray(b2), np.asarray(W3), np.asarray(b3), np.asarray(W4),
        np.asarray(b4),
    )
    in_maps = [
        {"xt": xt_per_core[c], "wpackr": wpackr, "wpackb": wpackb}
        for c in range(N_CORES)
    ]
    res = run_bass_kernel_spmd(nc, in_maps, list(range(N_CORES))).results
    mean = np.concatenate([res[c]["meant"] for c in range(N_CORES)], axis=1).T
    std = np.concatenate([res[c]["stdt"] for c in range(N_CORES)], axis=1).T
    return np.ascontiguousarray(mean), np.ascontiguousarray(std)
